# revision 1
# baseline (speedup 1.0000x reference)
"""ConvLSTM decoder Trainium2 kernel.

Strategy
--------
Data-parallel over batch: 64 images -> 8 NeuronCores x 8 images. Conv/dense
weights replicated on every core. The T=48 recurrence runs as a `For_i` loop
on-device; each step does two ConvLSTM layers.

Conv3x3(SAME) is computed as 9 shifted matmuls accumulating in PSUM:
  z[gate*128+m, img, y, x] = sum_{dy,dx,c} W[.., c, dy, dx] * in[c, img, y+dy-1, x+dx-1]
with the input planes stored zero-padded (17x17) in SBUF, channels on
partitions. Contraction over input channels (<=128 per chunk) maps to the PE
partition dim; each gate is exactly one 128-partition output chunk. Matmul
free dim = 2 images x 225 pixels = 450 (fits one PSUM bank).

All matmul operands are bf16 (fp32 PSUM accumulation); gate math + cell state
are fp32. Measured numerics vs the fp32 reference: ~0.35% L2 rel err.

Host-side prep (inside kernel()): shard the batch, zero-pad + cast x to bf16,
repack weights into lhsT layouts. Device does all conv/LSTM/dense compute.

NOTE on DMA count: walrus caps sync-wait commands per instruction; the For_i
back-edge drain waits on one sem per DMAHW lane ever used (round-robin over
8) plus one per engine. Keep the total number of dma_start calls <= 4 so only
lanes 0-3 exist: all weights ride in one DMA, all biases in another, x is one
DMA per step (same static instruction = same lane), output one.
"""

import numpy as np
import ml_dtypes

import concourse.bass as bass
from concourse import bacc
import concourse.mybir as mybir
import concourse.tile as tile
from concourse.bass import ds
from concourse.bass_utils import run_bass_kernel_spmd

BF16 = ml_dtypes.bfloat16
F32 = mybir.dt.float32
BF = mybir.dt.bfloat16

# Problem constants (hardcoded per contract).
B, T, C_IN, H, W = 64, 48, 64, 15, 15
HID, KK, OUT = 128, 3, 128
NCORES = 8
BC = B // NCORES          # images per core = 8
PH, PW = H + 2, W + 2     # padded plane 17x17
PP = PH * PW              # 289
ROW = BC * PP             # 2312 free elems in padded activations
S = H * W                 # 225
NPAIR = BC // 2           # 4 image pairs
NF = 2 * S                # 450 matmul free dim
WCONV = 9 * 2 * 512       # per-layer conv weight free size (9216)
WALL = 2 * WCONV + S * OUT  # 47232
AFT = mybir.ActivationFunctionType


def build_nc(t_steps: int = T) -> bass.Bass:
    nc = bacc.Bacc("TRN2", target_bir_lowering=False, debug=False)

    x_d = nc.dram_tensor("x", [t_steps * C_IN, ROW], BF, kind="ExternalInput")
    w_d = nc.dram_tensor("w", [128, WALL], BF, kind="ExternalInput")
    b_d = nc.dram_tensor("b", [128, 9], F32, kind="ExternalInput")
    out_d = nc.dram_tensor("out", [OUT, BC], F32, kind="ExternalOutput")

    # Persistent SBUF state.
    xsb = nc.alloc_sbuf_tensor("xsb", [128, ROW], BF)     # x_t padded (rows 64+ zero)
    h1p = nc.alloc_sbuf_tensor("h1p", [128, ROW], BF)     # layer-1 hidden, padded
    h2p = nc.alloc_sbuf_tensor("h2p", [128, ROW], BF)     # layer-2 hidden, padded
    c1 = nc.alloc_sbuf_tensor("c1", [128, BC * S], F32)
    c2 = nc.alloc_sbuf_tensor("c2", [128, BC * S], F32)
    wsb = nc.alloc_sbuf_tensor("wsb", [128, WALL], BF)    # w0 | w1 | wd
    bsb = nc.alloc_sbuf_tensor("bsb", [128, 9], F32)      # b0 | b1 | bd

    def padded(ap):
        return ap.rearrange("p (i y x) -> p i y x", i=BC, y=PH, x=PW)

    w0v = wsb.ap()[:, 0:WCONV].rearrange("p (t k o) -> p t k o", t=9, k=2, o=512)
    w1v = wsb.ap()[:, WCONV:2 * WCONV].rearrange("p (t k o) -> p t k o", t=9, k=2, o=512)
    wdv = wsb.ap()[:, 2 * WCONV:].rearrange("p (s o) -> p s o", s=S, o=OUT)

    # The walrus codegen caps sync-wait commands per instruction (~4-5); a
    # For_i back-edge drain waits on one sem per engine + one per DMA lane
    # used anywhere in its TileContext. Split the program into three
    # sequential TileContexts so each context's drains see few sems: the
    # loop context contains exactly one DMA instruction (the x load).
    with tile.TileContext(nc) as tc:
        # --- preamble: zero state + load weights ---
        nc.vector.memset(xsb.ap()[64:128, :], 0.0)
        nc.vector.memset(h1p.ap()[:, :], 0.0)
        nc.vector.memset(h2p.ap()[:, :], 0.0)
        nc.vector.memset(c1.ap()[:, :], 0.0)
        nc.vector.memset(c2.ap()[:, :], 0.0)
        nc.sync.dma_start(wsb.ap()[:, :], w_d.ap()[:, :])
        nc.sync.dma_start(bsb.ap()[:, :], b_d.ap()[:, :])

    with tile.TileContext(nc) as tc:
        with (
            tc.tile_pool(name="psum", bufs=8, space="PSUM") as psum,
            tc.tile_pool(name="gates", bufs=5) as gates,
            tc.tile_pool(name="tmps", bufs=3) as tmps,
        ):
            def lstm_layer(inp, selfp, cst, wv, bofs):
                """One ConvLSTM step. inp = input planes (kc=0 weights),
                selfp = this layer's hidden planes (kc=1); writes selfp."""
                inv = padded(inp.ap())
                selfv = padded(selfp.ap())
                gsb = []
                for g in range(4):
                    pts = [psum.tile([128, NF], F32, tag="ps", name=f"ps{g}_{i}")
                           for i in range(NPAIR)]
                    for kc, src in ((1, selfv), (0, inv)):
                        for tap in range(9):
                            dy, dx = divmod(tap, 3)
                            lhsT = wv[:, tap, kc, g * 128:(g + 1) * 128]
                            for ip in range(NPAIR):
                                rhs = src[:, 2 * ip:2 * ip + 2,
                                          dy:dy + H, dx:dx + W]
                                nc.tensor.matmul(
                                    pts[ip][:, :], lhsT, rhs,
                                    start=(kc == 1 and tap == 0),
                                    stop=(kc == 0 and tap == 8),
                                )
                    gt = gates.tile([128, BC * S], F32, tag="gate", name=f"g{g}")
                    func = AFT.Tanh if g == 3 else AFT.Sigmoid
                    for ip in range(NPAIR):
                        nc.scalar.activation(
                            gt[:, ip * NF:(ip + 1) * NF], pts[ip][:, :],
                            func, bias=bsb.ap()[:, bofs + g:bofs + g + 1])
                    gsb.append(gt)
                gi, gf, go, gg = gsb
                t1 = tmps.tile([128, BC * S], F32, tag="tmp", name="t1")
                t2 = tmps.tile([128, BC * S], F32, tag="tmp", name="t2")
                tch = tmps.tile([128, BC * S], F32, tag="tmp", name="tch")
                nc.vector.tensor_mul(t1[:, :], gf[:, :], cst.ap()[:, :])
                nc.vector.tensor_mul(t2[:, :], gi[:, :], gg[:, :])
                nc.vector.tensor_add(cst.ap()[:, :], t1[:, :], t2[:, :])
                nc.scalar.activation(tch[:, :], cst.ap()[:, :], AFT.Tanh)
                hdst = selfv[:, :, 1:1 + H, 1:1 + W]
                ov = go[:, :].rearrange("p (i y x) -> p i y x", i=BC, y=H, x=W)
                tv = tch[:, :].rearrange("p (i y x) -> p i y x", i=BC, y=H, x=W)
                nc.vector.tensor_mul(hdst, ov, tv)

            x2 = x_d.ap()
            with tc.For_i(0, t_steps * C_IN, C_IN) as iv:
                nc.sync.dma_start(xsb.ap()[0:C_IN, :], x2[ds(iv, C_IN), :])
                lstm_layer(xsb, h1p, c1, w0v, 0)
                lstm_layer(h1p, h2p, c2, w1v, 4)

    with tile.TileContext(nc) as tc:
        with (
            tc.tile_pool(name="psum2", bufs=1, space="PSUM") as psum2,
            tc.tile_pool(name="outp", bufs=1) as outp,
        ):
            # Dense head: out[o, img] = sum_{c,s} h2[c, img, s] * Wd[c*225+s, o]
            h2v = padded(h2p.ap())
            po = psum2.tile([128, BC], F32, tag="ps", name="po")
            for s in range(S):
                py, px = divmod(s, 15)
                rhs = h2v[:, :, 1 + py, 1 + px]
                nc.tensor.matmul(po[:, :], wdv[:, s, :], rhs,
                                 start=(s == 0), stop=(s == S - 1))
            osb = outp.tile([128, BC], F32, tag="o", name="osb")
            nc.scalar.activation(osb[:, :], po[:, :], AFT.Identity,
                                 bias=bsb.ap()[:, 8:9])
            nc.sync.dma_start(out_d.ap()[:, :], osb[:, :])

    nc.compile()
    return nc


def pack_inputs(inputs: dict, t_steps: int = T) -> tuple[list[dict], dict]:
    """Host-side layout prep. Returns (per_core_in_maps, shared_tensors)."""
    enc = np.ascontiguousarray(np.asarray(inputs["encoder_output"], np.float32))
    W0 = np.asarray(inputs["W0"], np.float32)
    W1 = np.asarray(inputs["W1"], np.float32)
    b0 = np.asarray(inputs["b0"], np.float32)
    b1 = np.asarray(inputs["b1"], np.float32)
    Wd = np.asarray(inputs["Wd"], np.float32)
    bd = np.asarray(inputs["bd"], np.float32)

    def pack_conv(Wc, cin0):
        # Wc: [512, cin0+128, 3, 3] -> [128, 9*2*512] (k, (tap, kchunk, o))
        Wr = Wc.reshape(512, Wc.shape[1], 9)
        w = np.zeros((128, 9, 2, 512), np.float32)
        w[:cin0, :, 0, :] = Wr[:, :cin0].transpose(1, 2, 0)
        w[:, :, 1, :] = Wr[:, cin0:cin0 + 128].transpose(1, 2, 0)
        return w.reshape(128, WCONV)

    wall = np.concatenate(
        [pack_conv(W0, C_IN), pack_conv(W1, HID), Wd.reshape(HID, S * OUT)],
        axis=1).astype(BF16)
    ball = np.concatenate(
        [b0.reshape(4, 128).T, b1.reshape(4, 128).T, bd.reshape(128, 1)],
        axis=1).astype(np.float32)
    ball = np.ascontiguousarray(ball)

    shared = {"w": wall, "b": ball}
    in_maps = []
    for c in range(NCORES):
        xc = enc[c * BC:(c + 1) * BC, :t_steps]          # [8, t, 64, 15, 15]
        xp = np.zeros((t_steps, C_IN, BC, PH, PW), BF16)
        xp[:, :, :, 1:1 + H, 1:1 + W] = xc.transpose(1, 2, 0, 3, 4)
        in_maps.append({"x": xp.reshape(t_steps * C_IN, ROW), **shared})
    return in_maps, shared


def kernel(**inputs) -> np.ndarray:
    nc = build_nc(T)
    in_maps, _ = pack_inputs(inputs, T)
    res = run_bass_kernel_spmd(nc, in_maps, list(range(NCORES))).results
    out = np.concatenate([np.asarray(r["out"], np.float32).T for r in res], axis=0)
    return np.ascontiguousarray(out)


if __name__ == "__main__":
    ins = {k: np.asarray(v) for k, v in np.load("inputs.npz").items()}
    out = kernel(**ins)
    exp = np.load("expected.npy")
    d = out - exp
    print("rel l2:", np.linalg.norm(d) / np.linalg.norm(exp))



# revision 3
# speedup vs baseline: 1.0083x; 1.0083x over previous
"""ConvLSTM decoder Trainium2 kernel, v2.

Strategy (v2 changes over v1)
-----------------------------
- Fully unrolled T=48 loop (no For_i back-edge barriers -> cross-step
  overlap; pointwise chains hide under the other layer's matmuls).
- Layer-1 x-side tap packing: x has only 64 channels, so two taps ride in
  one 128-partition contraction chunk. Host ships x2 = [x ; x shifted one
  col] and x3 = [x ; x shifted one row]; taps (dy,0)+(dy,1) pair via x2,
  (0,2)+(1,2) pair via x3, (2,2) runs alone on 64 partitions.
  L1: 9 h-taps + 5 x-passes = 14 passes/gate (was 18). L2: 18.
- Pair-blocked PSUM: per gate one [128, 4*512] f32 tile (4 banks, pair ip
  at col ip*512), drained by ONE activation over a strided [128,4,450] AP.
  2 tiles rotate (8 banks total).
- Emission order per layer: g0-self, g1-self, g0-in, g1-in, ACTs, then
  g2/g3 likewise, so layer-2's self (h2) taps run while layer-1's
  pointwise chain produces h1.
- Dense head: lhsT = h2 pixel slice [128c, 8img] (8-col weight loads),
  rhs = Wd slice [128c, 128o]; 225 matmuls N=128. Bias bd added on host.

Numerics: bf16 matmul operands, fp32 PSUM/gates/cell state (fp8 was
measured at 3-4% final error vs the 2% budget -- rejected).
"""

import numpy as np
import ml_dtypes

import concourse.bass as bass
from concourse import bacc
import concourse.mybir as mybir
import concourse.tile as tile
from concourse.bass_utils import run_bass_kernel_spmd

BF16 = ml_dtypes.bfloat16
F32 = mybir.dt.float32
BF = mybir.dt.bfloat16

B, T, C_IN, H, W = 64, 48, 64, 15, 15
HID, KK, OUT = 128, 3, 128
NCORES = 8
BC = B // NCORES          # images per core = 8
PH, PW = H + 2, W + 2     # padded plane 17x17
PP = PH * PW              # 289
ROW = BC * PP             # 2312
S = H * W                 # 225
NPAIR = BC // 2           # 4
NF = 2 * S                # 450 matmul free dim
NP1, NP2 = 14, 18         # passes per gate, layers 1 and 2
W1COLS = NP1 * 4 * 128    # 7168
W2COLS = NP2 * 4 * 128    # 9216
WDCOLS = S * OUT          # 28800
WTOT = W1COLS + W2COLS + WDCOLS
AFT = mybir.ActivationFunctionType


def build_nc(t_steps: int = T) -> bass.Bass:
    nc = bacc.Bacc("TRN2", target_bir_lowering=False, debug=False)

    x_d = nc.dram_tensor("x", [t_steps * 128, 2 * ROW], BF, kind="ExternalInput")
    w_d = nc.dram_tensor("w", [128, WTOT], BF, kind="ExternalInput")
    b_d = nc.dram_tensor("b", [128, 8], F32, kind="ExternalInput")
    out_d = nc.dram_tensor("out", [BC, OUT], F32, kind="ExternalOutput")

    h1p = nc.alloc_sbuf_tensor("h1p", [128, ROW], BF)
    h2p = nc.alloc_sbuf_tensor("h2p", [128, ROW], BF)
    c1 = nc.alloc_sbuf_tensor("c1", [128, BC * S], F32)
    c2 = nc.alloc_sbuf_tensor("c2", [128, BC * S], F32)
    wsb = nc.alloc_sbuf_tensor("wsb", [128, WTOT], BF)
    bsb = nc.alloc_sbuf_tensor("bsb", [128, 8], F32)

    def planes(ap):
        return ap.rearrange("p (i y x) -> p i y x", i=BC, y=PH, x=PW)

    wl1 = wsb.ap()[:, 0:W1COLS].rearrange(
        "p (ps g m) -> p ps g m", ps=NP1, g=4, m=128)
    wl2 = wsb.ap()[:, W1COLS:W1COLS + W2COLS].rearrange(
        "p (ps g m) -> p ps g m", ps=NP2, g=4, m=128)
    wdv = wsb.ap()[:, W1COLS + W2COLS:].rearrange(
        "p (s o) -> p s o", s=S, o=OUT)

    with tile.TileContext(nc) as tc:
        nc.vector.memset(h1p.ap()[:, :], 0.0)
        nc.vector.memset(h2p.ap()[:, :], 0.0)
        nc.vector.memset(c1.ap()[:, :], 0.0)
        nc.vector.memset(c2.ap()[:, :], 0.0)
        nc.sync.dma_start(wsb.ap()[:, :], w_d.ap()[:, :])
        nc.sync.dma_start(bsb.ap()[:, :], b_d.ap()[:, :])

        with (
            tc.tile_pool(name="psum", bufs=2, space="PSUM") as psum,
            tc.tile_pool(name="gates", bufs=5) as gates,
            tc.tile_pool(name="tmps", bufs=2) as tmps,
            tc.tile_pool(name="xin", bufs=2) as xin,
        ):
            h1v, h2v = planes(h1p.ap()), planes(h2p.ap())

            def l1_passes(xt):
                """(lhsT-pass-idx, rhs_window_fn, n_part) for layer 1."""
                x2 = xt[:, 0:ROW].rearrange(
                    "p (i y x) -> p i y x", i=BC, y=PH, x=PW)
                x3 = xt[:, ROW:2 * ROW].rearrange(
                    "p (i y x) -> p i y x", i=BC, y=PH, x=PW)
                ps = []
                for tap in range(9):
                    dy, dx = divmod(tap, 3)
                    ps.append((lambda ip, dy=dy, dx=dx:
                               h1v[:, 2 * ip:2 * ip + 2, dy:dy + H, dx:dx + W],
                               128))
                for dy in range(3):
                    ps.append((lambda ip, dy=dy:
                               x2[:, 2 * ip:2 * ip + 2, dy:dy + H, 0:W], 128))
                ps.append((lambda ip: x3[:, 2 * ip:2 * ip + 2, 0:H, 2:2 + W],
                           128))
                ps.append((lambda ip: x2[0:64, 2 * ip:2 * ip + 2, 2:2 + H,
                                         2:2 + W], 64))
                return ps

            def l2_passes():
                ps = []
                for src in (h2v, h1v):
                    for tap in range(9):
                        dy, dx = divmod(tap, 3)
                        ps.append((lambda ip, dy=dy, dx=dx, src=src:
                                   src[:, 2 * ip:2 * ip + 2, dy:dy + H,
                                       dx:dx + W], 128))
                return ps

            def gate_mms(wv, passes, g):
                pt = psum.tile([128, 4 * 512], F32, tag="ps", name=f"ps{g}")
                np_ = len(passes)
                for pi, (rhs_fn, npart) in enumerate(passes):
                    lhsT = wv[0:npart, pi, g, :]
                    for ip in range(NPAIR):
                        nc.tensor.matmul(
                            pt[:, ip * 512:ip * 512 + NF], lhsT, rhs_fn(ip),
                            start=(pi == 0), stop=(pi == np_ - 1))
                return pt

            def gate_act(pt, g, bofs):
                gt = gates.tile([128, BC * S], F32, tag="gate", name=f"g{g}")
                func = AFT.Tanh if g == 3 else AFT.Sigmoid
                nc.scalar.activation(
                    gt.rearrange("p (i n) -> p i n", i=NPAIR),
                    pt.rearrange("p (i n) -> p i n", i=NPAIR)[:, :, 0:NF],
                    func, bias=bsb.ap()[:, bofs + g:bofs + g + 1])
                return gt

            def lstm_layer(passes, wv, bofs, cst, selfv):
                gsb = [None] * 4
                for gpair in ((0, 1), (2, 3)):
                    pts = {}
                    for g in gpair:          # self taps first (ready early)
                        pts[g] = gate_mms(wv, passes, g)
                    for g in gpair:
                        gsb[g] = gate_act(pts[g], g, bofs)
                gi, gf, go, gg = gsb
                t1 = tmps.tile([128, BC * S], F32, tag="tmp", name="t1")
                t2 = tmps.tile([128, BC * S], F32, tag="tmp", name="t2")
                nc.vector.tensor_mul(t1[:, :], gf[:, :], cst.ap()[:, :])
                nc.vector.tensor_mul(t2[:, :], gi[:, :], gg[:, :])
                nc.vector.tensor_add(cst.ap()[:, :], t1[:, :], t2[:, :])
                tch = tmps.tile([128, BC * S], F32, tag="tmp", name="tch")
                nc.scalar.activation(tch[:, :], cst.ap()[:, :], AFT.Tanh)
                hdst = selfv[:, :, 1:1 + H, 1:1 + W]
                ov = go[:, :].rearrange("p (i y x) -> p i y x", i=BC, y=H, x=W)
                tv = tch[:, :].rearrange("p (i y x) -> p i y x", i=BC, y=H, x=W)
                nc.vector.tensor_mul(hdst, ov, tv)

            ps2 = l2_passes()
            for t in range(t_steps):
                xt = xin.tile([128, 2 * ROW], BF, tag="x", name=f"x{t}")
                nc.sync.dma_start(xt[:, :],
                                  x_d.ap()[t * 128:(t + 1) * 128, :])
                lstm_layer(l1_passes(xt), wl1, 0, c1, h1v)
                lstm_layer(ps2, wl2, 4, c2, h2v)

        # Dense head: out[img, o] = sum_{c,s} h2[c, img, s] * Wd[(c,s), o]
        with (
            tc.tile_pool(name="psum2", bufs=1, space="PSUM") as psum2,
            tc.tile_pool(name="outp", bufs=1) as outp,
        ):
            po = psum2.tile([BC, OUT], F32, tag="po", name="po")
            for s in range(S):
                py, px = divmod(s, W)
                lhsT = h2v[:, :, 1 + py, 1 + px]
                nc.tensor.matmul(po[:, :], lhsT, wdv[:, s, :],
                                 start=(s == 0), stop=(s == S - 1))
            osb = outp.tile([BC, OUT], F32, tag="o", name="osb")
            nc.vector.tensor_copy(osb[:, :], po[:, :])
            nc.sync.dma_start(out_d.ap()[:, :], osb[:, :])

    nc.compile()
    return nc


def pack_inputs(inputs: dict, t_steps: int = T) -> tuple[list[dict], dict]:
    enc = np.ascontiguousarray(np.asarray(inputs["encoder_output"], np.float32))
    W0 = np.asarray(inputs["W0"], np.float32)
    W1 = np.asarray(inputs["W1"], np.float32)
    b0 = np.asarray(inputs["b0"], np.float32)
    b1 = np.asarray(inputs["b1"], np.float32)
    Wd = np.asarray(inputs["Wd"], np.float32)

    # --- weights ---
    # layer 1: wl1[c, pass, gate, m]; gate g covers out channels g*128+m
    wx = W0[:, :C_IN]          # [512, 64, 3, 3]
    wh = W0[:, C_IN:]          # [512, 128, 3, 3]
    w1 = np.zeros((128, NP1, 4, 128), np.float32)
    W0g = wh.reshape(4, 128, 128, 3, 3)     # [g, m, c, dy, dx]
    Wxg = wx.reshape(4, 128, C_IN, 3, 3)
    for tap in range(9):
        dy, dx = divmod(tap, 3)
        w1[:, tap] = W0g[:, :, :, dy, dx].transpose(2, 0, 1)
    for dy in range(3):
        w1[0:64, 9 + dy] = Wxg[:, :, :, dy, 0].transpose(2, 0, 1)
        w1[64:128, 9 + dy] = Wxg[:, :, :, dy, 1].transpose(2, 0, 1)
    w1[0:64, 12] = Wxg[:, :, :, 0, 2].transpose(2, 0, 1)
    w1[64:128, 12] = Wxg[:, :, :, 1, 2].transpose(2, 0, 1)
    w1[0:64, 13] = Wxg[:, :, :, 2, 2].transpose(2, 0, 1)

    # layer 2: self (h2) taps 0-8 then input (h1) taps 9-17
    wh1 = W1[:, :HID].reshape(4, 128, 128, 3, 3)
    wh2 = W1[:, HID:].reshape(4, 128, 128, 3, 3)
    w2 = np.zeros((128, NP2, 4, 128), np.float32)
    for tap in range(9):
        dy, dx = divmod(tap, 3)
        w2[:, tap] = wh2[:, :, :, dy, dx].transpose(2, 0, 1)
        w2[:, 9 + tap] = wh1[:, :, :, dy, dx].transpose(2, 0, 1)

    wall = np.concatenate(
        [w1.reshape(128, W1COLS), w2.reshape(128, W2COLS),
         Wd.reshape(HID, S * OUT)], axis=1).astype(BF16)
    ball = np.concatenate(
        [b0.reshape(4, 128).T, b1.reshape(4, 128).T], axis=1).astype(np.float32)
    ball = np.ascontiguousarray(ball)

    shared = {"w": wall, "b": ball}
    in_maps = []
    for c in range(NCORES):
        xc = enc[c * BC:(c + 1) * BC, :t_steps]      # [8, t, 64, 15, 15]
        xp = np.zeros((t_steps, C_IN, BC, PH, PW), np.float32)
        xp[:, :, :, 1:1 + H, 1:1 + W] = xc.transpose(1, 2, 0, 3, 4)
        xfull = np.zeros((t_steps, 128, 2, BC, PH, PW), np.float32)
        xfull[:, 0:64, 0] = xp                       # x2 top: x
        xfull[:, 64:128, 0, :, :, :-1] = xp[..., 1:]  # x2 bot: col-shift
        xfull[:, 0:64, 1] = xp                       # x3 top: x
        xfull[:, 64:128, 1, :, :-1, :] = xp[:, :, :, 1:, :]  # x3 bot: row-shift
        in_maps.append({"x": xfull.astype(BF16).reshape(t_steps * 128, 2 * ROW),
                        **shared})
    return in_maps, shared


def unpack_output(results, inputs) -> np.ndarray:
    bd = np.asarray(inputs["bd"], np.float32)
    out = np.concatenate(
        [np.asarray(r["out"], np.float32) for r in results], axis=0)
    return np.ascontiguousarray(out + bd[None, :])


def kernel(**inputs) -> np.ndarray:
    nc = build_nc(T)
    in_maps, _ = pack_inputs(inputs, T)
    res = run_bass_kernel_spmd(nc, in_maps, list(range(NCORES))).results
    return unpack_output(res, inputs)
